# revision 2
# baseline (speedup 1.0000x reference)
"""Trainium2 Bass kernel for nn_GeneralizedAttention (Performer-style linear
attention with GELU random features) — modulo-scheduled pipeline v2.

Math (per (b,h)):
    qp  = gelu(q @ proj^T)            [n, m]
    kp  = gelu(k @ proj^T)            [n, m]
    ctxT= kp^T @ [v | 1]              [m, e+1]  (col e = ksum)
    out = (qp @ ctxT[:, :e]) / (qp @ ctxT[:, e])[:, None]

Sharding: B*H = 64 pairs split across 8 cores, 8 pairs each; proj replicated.

n-index mapping (all tensors consistently):  n = c*512 + p*4 + r
(c in 0..7, p = SBUF partition 0..127, r in 0..3) so every q/k/out DMA
descriptor moves >= 512B contiguous.

Per-core modulo schedule, 16 "slots" per (b,h) phase j — one ACT gelu group
per slot (ACT is the bottleneck engine and runs back-to-back):
  slots 0-7   kp score matmuls (c=slot) + gelu     | fillers: ctx(j-1) s0-3,
  slots 8-15  qp score matmuls + gelu              |   fin(j-1)+store s4-7,
                                                   |   transposes(j+1) s8-11,
                                                   |   loads(j+2) s12
All phase indices are mod 8, so the pipeline wraps the repeat-loop boundary
seamlessly: the prologue (loads/transposes of bh0/1) runs once before the
loop, and the epilogue (ctx/fin/store of bh7, which the body defers to the
next iteration) runs once after it. Iteration 0's wrapped fillers read
uninitialized buffers and store garbage to out[7], which the epilogue then
overwrites with the correct values.

ctx/fin run in the weight-stationary orientation (f=65 streams) so no output
transposes are needed: fin produces out[n-chunk, e+1] directly (col 64 = den).
"""

import numpy as np

B, H, N, D, M = 4, 16, 4096, 64, 256
NCORES = 8
BH = B * H
BHPC = BH // NCORES
P = 128
NC = 8          # c blocks
NR = 4          # r per partition
EAUG = D + 1    # 65


def _emit_body(ctx, tc, out_d, q_d, k_d, v_d, proj_d, bhpc, repeat=1, unroll=1):
    import concourse.bass as bass
    import concourse.mybir as mybir
    from concourse.masks import make_identity

    nc = tc.nc
    f32 = mybir.dt.float32
    bf16 = mybir.dt.bfloat16
    MULT = mybir.AluOpType.mult
    GELU = mybir.ActivationFunctionType.Gelu

    const = ctx.enter_context(tc.tile_pool(name="const", bufs=1))
    inp = ctx.enter_context(tc.tile_pool(name="inp", bufs=6))
    vpool = ctx.enter_context(tc.tile_pool(name="vpool", bufs=4))
    tsb = ctx.enter_context(tc.tile_pool(name="tsb", bufs=4))
    feat = ctx.enter_context(tc.tile_pool(name="feat", bufs=2))
    small = ctx.enter_context(tc.tile_pool(name="small", bufs=2))
    recp = ctx.enter_context(tc.tile_pool(name="recp", bufs=4))
    outp = ctx.enter_context(tc.tile_pool(name="outp", bufs=2))
    ps_act = ctx.enter_context(tc.tile_pool(name="ps_act", bufs=2, space="PSUM"))
    ps_tr = ctx.enter_context(tc.tile_pool(name="ps_tr", bufs=1, space="PSUM"))
    ps_fin = ctx.enter_context(tc.tile_pool(name="ps_fin", bufs=2, space="PSUM"))
    ps_ctx = ctx.enter_context(tc.tile_pool(name="ps_ctx", bufs=1, space="PSUM"))

    ident_bf = const.tile([P, P], bf16, name="ident_bf")
    make_identity(nc, ident_bf)
    ident_f32 = const.tile([P, P], f32, name="ident_f32")
    make_identity(nc, ident_f32)

    # proj^T [d, m] duplicated on both partition halves (rows 0-63 == 64-127)
    proj_nat = const.tile([P, 2, D], f32, name="proj_nat")
    nc.sync.dma_start(proj_nat[:], proj_d.rearrange("(t p) d -> p t d", p=P))
    projT = const.tile([P, M], bf16, name="projT")
    for t in range(2):
        pspt = ps_tr.tile([D, P], f32, tag="tr", name=f"ps_projT{t}")
        nc.tensor.transpose(pspt[:], proj_nat[:, t, :], ident_f32)
        nc.vector.tensor_copy(projT[0:D, P * t : P * (t + 1)], pspt[:])
        nc.vector.tensor_copy(projT[D:P, P * t : P * (t + 1)], pspt[:])

    # ---- per-phase tile handles, allocated up front in phase-WRITE order so
    # the pool rings rotate exactly one write-read generation apart ----
    qt, kt, va, qT, kT, kp, qpT = ({} for _ in range(7))
    for j in range(bhpc):
        qt[j] = inp.tile([P, NC, NR, D], bf16, tag="qk", name=f"q_tile{j}")
        kt[j] = inp.tile([P, NC, NR, D], bf16, tag="qk", name=f"k_tile{j}")
    for j in range(bhpc):
        va[j] = vpool.tile([P, NC, NR, EAUG], bf16, tag="va", name=f"v_aug{j}")
    for j in range(bhpc):
        qT[j] = tsb.tile([P, 2, NC, P], bf16, tag="t", name=f"qT{j}")
        kT[j] = tsb.tile([P, 2, NC, P], bf16, tag="t", name=f"kT{j}")
    for j in range(bhpc):
        kp[j] = feat.tile([P, NC, NR, M], bf16, tag="kp", name=f"kp{j}")
        qpT[j] = feat.tile([P, 2, NR, NC, P], bf16, tag="qpT", name=f"qpT{j}")

    csb = [None] * bhpc   # ctx_sb [128, 2, EAUG] bf16 (use-site alloc)
    ost = [None] * bhpc   # out_stage [128, NC, NR, D] f32 (use-site alloc)
    pctx = [None] * bhpc  # ps_ctx tile (use-site alloc)

    def emit_loads(j):
        nc.gpsimd.dma_start(
            qt[j][:], q_d[j].rearrange("(c p r) d -> p c r d", p=P, r=NR)
        )
        nc.gpsimd.dma_start(
            kt[j][:], k_d[j].rearrange("(c p r) d -> p c r d", p=P, r=NR)
        )
        nc.gpsimd.memset(va[j][:, :, :, D:EAUG], 1.0)
        v_src = v_d[j].rearrange("(c p r) d -> p c r d", p=P, r=NR)
        for r in range(NR):
            nc.gpsimd.dma_start(va[j][:, :, r, 0:D], v_src[:, :, r, :])

    def emit_transpose_group(j, g):
        # g = 0..3 -> (q, h=0), (q, h=1), (k, h=0), (k, h=1)
        src = qt[j] if g < 2 else kt[j]
        dst = qT[j] if g < 2 else kT[j]
        h = g % 2
        pst = ps_tr.tile([P, NC, P], bf16, tag="tr", name=f"ps_tr{j}g{g}")
        for c in range(NC):
            nc.tensor.transpose(
                pst[:, c, :],
                src[:, c, 2 * h : 2 * h + 2, :].rearrange("p r d -> p (r d)"),
                ident_bf,
            )
        nc.vector.tensor_copy(dst[:, h, :, :], pst[:])

    def emit_kp_group(j, c):
        # psum region (r%2)*2 + r//2 so consecutive matmuls land in
        # alternating PSUM banks — back-to-back non-accumulating matmuls
        # into the same bank fault on hardware
        psk = ps_act.tile([P, NR, M], f32, tag="act", name=f"ps_kp{j}_{c}")
        for r in range(NR):
            h, rh = r // 2, r % 2
            g = (r % 2) * 2 + r // 2
            nc.tensor.matmul(
                psk[:, g, :],
                lhsT=kT[j][64 * rh : 64 * rh + 64, h, c, :],
                rhs=projT[64 * rh : 64 * rh + 64, :],
            )
        nc.scalar.activation(
            kp[j][:, c, :, :].rearrange("p (a b) m -> p a b m", a=2),
            psk.rearrange("p (b a) m -> p a b m", b=2),
            GELU,
        )

    def emit_qp_group(j, idx):
        # idx = 0..7 -> (mc, h, cq)
        mc, h, cq = idx // 4, (idx // 2) % 2, idx % 2
        psq = ps_act.tile([P, 2, 4, P], f32, tag="act", name=f"ps_qp{j}_{idx}")
        for rh in range(2):
            nc.tensor.matmul(
                psq[:, rh, :, :],
                lhsT=projT[64 * rh : 64 * rh + 64, P * mc : P * (mc + 1)],
                rhs=qT[j][64 * rh : 64 * rh + 64, h, 4 * cq : 4 * cq + 4, :],
            )
        nc.scalar.activation(
            qpT[j][:, mc, 2 * h : 2 * h + 2, 4 * cq : 4 * cq + 4, :], psq[:], GELU
        )

    def emit_ctx_quarter(j, s):
        # s = 0..3 ; mc = s//2 ; mc-major so each accumulation group
        # occupies its bank region sequentially
        mc, half = s // 2, s % 2
        if s == 0:
            # row stride padded to 128 f32 so matmul PSUM targets stay
            # 512B-aligned within the bank
            pctx[j] = ps_ctx.tile([P, 2, P], f32, tag="ctx", name=f"ps_ctx{j}")
        for k in range(16):
            cr = 16 * half + k
            c, r = cr // NR, cr % NR
            nc.tensor.matmul(
                pctx[j][:, mc, 0:EAUG],
                lhsT=kp[j][:, c, r, P * mc : P * (mc + 1)],
                rhs=va[j][:, c, r, :],
                start=(cr == 0),
                stop=(cr == 31),
            )

    def emit_ctx_copy(j):
        csb[j] = small.tile([P, 2, EAUG], bf16, tag="csb", name=f"ctx_sb{j}")
        nc.vector.tensor_copy(csb[j][:], pctx[j][:, :, 0:EAUG])

    def emit_fin_group(j, c):
        if c == 0:
            ost[j] = outp.tile([P, NC, NR, D], f32, tag="ost", name=f"out_stage{j}")
        psf = ps_fin.tile([P, NR, P], f32, tag="fin", name=f"ps_fin{j}_{c}")
        for r in range(NR):
            for mc in range(2):
                nc.tensor.matmul(
                    psf[:, r, 0:EAUG],
                    lhsT=qpT[j][:, mc, r, c, :],
                    rhs=csb[j][:, mc, :],
                    start=(mc == 0),
                    stop=(mc == 1),
                )
        rec = recp.tile([P, NR], f32, tag="rec", name=f"rec{j}_{c}")
        nc.vector.reciprocal(rec[:], psf[:, :, D])
        nc.vector.tensor_tensor(
            ost[j][:, c, :, :],
            psf[:, :, 0:D],
            rec[:, :, None].to_broadcast((P, NR, D)),
            MULT,
        )

    def emit_store(j):
        nc.sync.dma_start(
            out_d[j].rearrange("(c p r) d -> p c r d", p=P, r=NR), ost[j][:]
        )

    # ---- prologue (once): seed bh0/bh1; zero the wrap-around buffers that
    # iteration 0's deferred-tail fillers read before anything wrote them ----
    jw = bhpc - 1
    nc.vector.memset(kp[jw][:], 0.0)
    nc.vector.memset(qpT[jw][:], 0.0)
    nc.gpsimd.memset(va[jw][:], 0.0)
    emit_loads(0)
    emit_loads(1)
    for g in range(4):
        emit_transpose_group(0, g)

    if repeat > 1:
        loop_cm = tc.For_i(0, repeat, 1)
        loop_cm.__enter__()

    # ---- steady-state body: all fillers unconditional, phases mod 8 ----
    for _ in range(unroll):
        for j in range(bhpc):
            jp = (j - 1) % bhpc
            jn = (j + 1) % bhpc
            jl = (j + 2) % bhpc
            for s in range(16):
                if s < 8:
                    emit_kp_group(j, s)
                else:
                    emit_qp_group(j, s - 8)
                if s < 4:
                    emit_ctx_quarter(jp, s)
                elif s == 4:
                    emit_ctx_copy(jp)
                if 4 <= s < 8:
                    emit_fin_group(jp, 2 * (s - 4))
                    emit_fin_group(jp, 2 * (s - 4) + 1)
                    if s == 7:
                        emit_store(jp)
                if 8 <= s < 12:
                    emit_transpose_group(jn, s - 8)
                if s == 12:
                    emit_loads(jl)

    if repeat > 1:
        loop_cm.__exit__(None, None, None)

    # ---- epilogue (once): the body deferred bh7's tail to the "next"
    # iteration; recompute it for real (also overwrites iteration-0's
    # garbage store to out[7]) ----
    jl = bhpc - 1
    for s in range(4):
        emit_ctx_quarter(jl, s)
    emit_ctx_copy(jl)
    for c in range(NC):
        emit_fin_group(jl, c)
    emit_store(jl)


def build(bhpc=BHPC, repeat=1, unroll=1):
    from contextlib import ExitStack

    import concourse.mybir as mybir
    import concourse.tile as tile
    from concourse import bacc

    nc = bacc.Bacc("TRN2", target_bir_lowering=False, debug=False)
    f32 = mybir.dt.float32
    q_d = nc.dram_tensor("q", [bhpc, N, D], f32, kind="ExternalInput").ap()
    k_d = nc.dram_tensor("k", [bhpc, N, D], f32, kind="ExternalInput").ap()
    v_d = nc.dram_tensor("v", [bhpc, N, D], f32, kind="ExternalInput").ap()
    proj_d = nc.dram_tensor("proj_mat", [M, D], f32, kind="ExternalInput").ap()
    out_d = nc.dram_tensor("out", [bhpc, N, D], f32, kind="ExternalOutput").ap()

    with tile.TileContext(nc) as tc:
        with ExitStack() as body_ctx:
            _emit_body(
                body_ctx, tc, out_d, q_d, k_d, v_d, proj_d, bhpc, repeat, unroll
            )
    nc.compile()
    return nc


_built = None


def _get_built():
    global _built
    if _built is None:
        _built = build()
    return _built


def _shard_inputs(q, k, v, proj_mat):
    qf = np.ascontiguousarray(q.reshape(BH, N, D), dtype=np.float32)
    kf = np.ascontiguousarray(k.reshape(BH, N, D), dtype=np.float32)
    vf = np.ascontiguousarray(v.reshape(BH, N, D), dtype=np.float32)
    pf = np.ascontiguousarray(proj_mat, dtype=np.float32)
    in_maps = []
    for c in range(NCORES):
        s = slice(c * BHPC, (c + 1) * BHPC)
        in_maps.append({"q": qf[s], "k": kf[s], "v": vf[s], "proj_mat": pf})
    return in_maps


def run_on_hw(q, k, v, proj_mat, trace=False, **kwargs):
    from concourse.bass_utils import run_bass_kernel_spmd

    nc = _get_built()
    in_maps = _shard_inputs(q, k, v, proj_mat)
    res = run_bass_kernel_spmd(
        nc, in_maps, core_ids=list(range(NCORES)), trace=trace, **kwargs
    )
    out = np.concatenate([r["out"] for r in res.results], axis=0)
    return out.reshape(B, H, N, D).astype(np.float32), res


def kernel(q, k, v, proj_mat):
    out, _ = run_on_hw(q, k, v, proj_mat, trace=False)
    return out
